# revision 32
# baseline (speedup 1.0000x reference)
"""Trainium2 Bass kernel for LocalAttentionLayer.

Problem: B=4, N=2048, H=8 heads, D=64, DM=512 (f32)
  q/k/v = x @ W{q,k,v}; sim = scale * q k^T (per head); mask_k/mask_q -> big_neg;
  softmax over keys; out = (attn @ v) @ Wo + bo.

Sharding (8 cores): core = 2*b + g -> batch b (4-way) x head-group g (2-way,
4 heads each).  Each core computes its batch's projections for its 4 heads,
full attention for those heads, and a partial output projection with its
256-row slice of Wo.  Host sums the two partials per batch, adds bo, and
overwrites masked-q rows (reference semantics: fully-masked rows degenerate
to uniform attention = mean over all v rows = (mean_j x) @ Wv @ Wo + bo).

Pipeline layout (v7):
  - BOTH masks are exploited by gathering on the host:
    * masked KEYS contribute exp() == 0 exactly -> gather valid keys first,
      run NJK = ceil(max_valid_k/128) j-tiles; pad keys keep the -1e5 bias.
    * masked QUERIES are overwritten by the host afterwards -> gather valid
      queries first, compute only NJQ = ceil(max_valid_q/128) i-tiles and
      scatter rows back on the host.
    Both cut exp work and QK/PV matmul rows by ~19% each.  The program is
    built per (njk, njq) and cached.
  - ACT is the bound (only engine with exp).  sim tiles [128 keys, <=1024 qs]
    live in a 2-deep PSUM pool; per j-section the PE issue order puts the
    next exp's QK before PV(j) so the in-order PE queue never head-of-line
    blocks ACT; the last section's QK belongs to the next round.
  - PV accumulators [65, icw] are CLAIMED LATE each round: early sections
    route side work (v / W projections, fo tiles) through the same PSUM
    slots; the PV backlog buffers in a deep pt pool and drains in PE slack.
  - Input DMA serializes at ~0.39ns per byte per partition, so ALL [DM, *]
    inputs ride ONE tensor (gathered xqT | gathered xkT | packed
    wq|wk|wva|bk) = 4 slice loads + 2 wo loads.
  - GPSIMD/Pool has no PSUM port on TRN2: every PSUM read is DVE or ACT
    (ACT only where it idles: pre-phase + tail).
  - Normalization: DVE reciprocal of the ones-column denominator row (bf16),
    gpsimd partition_broadcast (attn library) mid-kernel / PE broadcast in
    the tail, DVE multiply.
"""

import sys

if "/opt/trn_rl_repo" not in sys.path:
    sys.path.insert(0, "/opt/trn_rl_repo")

import ml_dtypes
import numpy as np

BF16 = np.dtype(ml_dtypes.bfloat16)

B, N, H, D = 4, 2048, 8, 64
DM = H * D  # 512
G = 2  # head-group split across cores
CG = DM // G  # 256 channels per group
HPG = H // G  # 4 heads per group
NJ = N // 128  # 16 full tiles
MASK_BIAS = -1.0e5
# packed weight block columns: wq | wk | wva | pad | bk(f32 as 2xbf16)
WIN_WQ = 0
WIN_WK = CG  # 256
WIN_WVA = 2 * CG  # 512
WIN_BK = 2 * CG + HPG * 65 + 32  # 804 (4B aligned for the f32 bitcast)

_NC_CACHE = {}


def _chunks(total, width=512):
    out = []
    c = 0
    while c < total:
        w = min(width, total - c)
        out.append((c, w))
        c += w
    return out


def _build_nc(njk, njq):
    from contextlib import ExitStack

    import concourse.bass as bass  # noqa: F401
    import concourse.mybir as mybir
    import concourse.tile as tile
    from concourse import bacc, library_config
    from concourse.bass import ts

    f32 = mybir.dt.float32
    bf16 = mybir.dt.bfloat16
    EXP = mybir.ActivationFunctionType.Exp

    nk = njk * 128  # gathered key count (padded)
    nq = njq * 128  # gathered query count (padded)
    win_cols = WIN_BK + 2 * njk
    # kT projection chunks: tiny first chunk (j-tile 0) unblocks the first
    # exp as early as possible
    kch = [(0, 128)] + _chunks(nk - 128)
    kch = [(c + (128 if i else 0), w) for i, (c, w) in enumerate(kch)]
    # i-chunks per round: ic0 = min(1024, nq), ic1 = rest
    IC0 = min(1024, nq)
    ICW = [IC0, nq - IC0] if nq > IC0 else [IC0]

    nc = bacc.Bacc(None, target_bir_lowering=False, debug=False)

    with tile.TileContext(nc) as tc, ExitStack() as ctx:
        dram = ctx.enter_context(tc.tile_pool(name="dram", bufs=1, space="DRAM"))
        const = ctx.enter_context(tc.tile_pool(name="const", bufs=1))
        ptp = ctx.enter_context(tc.tile_pool(name="ptp", bufs=24))
        rrp = ctx.enter_context(tc.tile_pool(name="rrp", bufs=2))
        # PSUM: psim tag "sim" 2 bufs x 2 banks + ppv tag "pv" 2 bufs x 2 banks
        psim = ctx.enter_context(tc.tile_pool(name="psim", bufs=2, space="PSUM"))
        ppv = ctx.enter_context(tc.tile_pool(name="ppv", bufs=2, space="PSUM"))

        # ---- DRAM I/O ----
        xall_cols = nq + nk + win_cols
        xall_d = dram.tile([DM, xall_cols], bf16, kind="ExternalInput", name="xall", uniquify=False)
        wo_d = dram.tile([CG, DM], bf16, kind="ExternalInput", name="wo", uniquify=False)
        out_d = dram.tile([nq, DM], f32, kind="ExternalOutput", name="out", uniquify=False)

        # ---- SBUF persistents ----
        xall_sb = [
            const.tile_from(xall_d[k * 128 : (k + 1) * 128, :], name=f"xa{k}")
            for k in range(4)
        ]
        wo_sb = [const.tile_from(wo_d[c * 128 : (c + 1) * 128, :], name=f"wos{c}") for c in range(2)]

        xT_sb = [xall_sb[k][:, 0:nq] for k in range(4)]
        xkT_sb = [xall_sb[k][:, nq : nq + nk] for k in range(4)]
        win0 = nq + nk

        def wq_ap(k):
            return xall_sb[k][:, win0 + WIN_WQ : win0 + WIN_WQ + CG]

        def wk_ap(k):
            return xall_sb[k][:, win0 + WIN_WK : win0 + WIN_WK + CG]

        def wva_ap(k):
            return xall_sb[k][:, win0 + WIN_WVA : win0 + WIN_WVA + HPG * 65]

        bk_sb = xall_sb[0][:, win0 + WIN_BK : win0 + WIN_BK + 2 * njk].bitcast(f32)

        # gpsimd "attn" library: partition_broadcast for the 1/denom rows
        nc.gpsimd.load_library(library_config.attn)

        # ones row for broadcasting 1/denom across 64 partitions via K=1
        # matmul in the tail (mid-kernel rounds use gpsimd broadcast)
        ones64 = const.tile([1, 64], bf16, name="ones64")
        nc.vector.memset(ones64[:, :], 1.0)

        # warm the ACT Exp table during the DMA phase so the first real exp
        # doesn't pay the table load
        warm_in = const.tile([1, 1], f32, name="warm_in")
        warm_out = const.tile([1, 1], f32, name="warm_out")
        nc.vector.memset(warm_in[:, :], 0.0)
        nc.scalar.activation(warm_out[:, :], warm_in[:, :], EXP, bias=0.0, scale=1.0)

        qT_sb = [const.tile([128, nq], bf16, name=f"qT{hp}") for hp in range(2)]
        kT_sb = [const.tile([128, nk], bf16, name=f"kT{hp}") for hp in range(2)]
        v_sb = [const.tile([128, HPG * 65], bf16, name=f"v{j}") for j in range(njk)]
        aT_sb = [const.tile([128, nq], bf16, name=f"aT{hp}") for hp in range(2)]

        # ---- building blocks ----
        qch = _chunks(nq)

        def q_chunk(hp, ci, copy_eng):
            c0, w = qch[ci]
            q_ps = ppv.tile([128, 512], f32, tag="pv", name="q_ps")
            for k in range(4):
                nc.tensor.matmul(
                    q_ps[:, 0:w],
                    wq_ap(k)[:, hp * 128 : (hp + 1) * 128],
                    xT_sb[k][:, c0 : c0 + w],
                    start=(k == 0),
                    stop=(k == 3),
                )
            copy_eng(qT_sb[hp][:, c0 : c0 + w], q_ps[:, 0:w])

        def k_chunk(hp, ci, copy_eng):
            c0, w = kch[ci]
            k_ps = ppv.tile([128, 512], f32, tag="pv", name="k_ps")
            for k in range(4):
                nc.tensor.matmul(
                    k_ps[:, 0:w],
                    wk_ap(k)[:, hp * 128 : (hp + 1) * 128],
                    xkT_sb[k][:, c0 : c0 + w],
                    start=(k == 0),
                    stop=(k == 3),
                )
            copy_eng(kT_sb[hp][:, c0 : c0 + w], k_ps[:, 0:w])

        # v: [j, c_aug] = xkT-slice.T @ wva; the per-head ones column (softmax
        # denominator rides the PV matmul) is a constant -> memset, no matmul
        def v_proj(j):
            v_ps = ppv.tile([128, HPG * 65], f32, tag="pv", name="v_ps")
            for k in range(4):
                nc.tensor.matmul(
                    v_ps[:, :],
                    xkT_sb[k][:, ts(j, 128)],
                    wva_ap(k),
                    start=(k == 0),
                    stop=(k == 3),
                )
            nc.vector.tensor_copy(v_sb[j][:, :], v_ps[:, :])
            nc.vector.memset(v_sb[j][:, 64 : HPG * 65 : 65], 1.0)

        # output projection for a group of 1-2 i-tiles -> one DMA
        def fo_group(tiles, copy_engs):
            fo_sb2 = ptp.tile([128, 1024], f32, tag="fo", bufs=4, name="fo_sb2")
            for half, it in enumerate(tiles):
                fo_ps = ppv.tile([128, 512], f32, tag="pv", name="fo_ps")
                for c in range(2):
                    nc.tensor.matmul(
                        fo_ps[:, :],
                        aT_sb[c][:, ts(it, 128)],
                        wo_sb[c][:, :],
                        start=(c == 0),
                        stop=(c == 1),
                    )
                copy_engs[half](fo_sb2[:, ts(half, 512)], fo_ps[:, :])
            it0 = tiles[0]
            nt = len(tiles)
            out_view = out_d.rearrange("(t pp) d -> pp t d", pp=128)[:, it0 : it0 + nt, :]
            in_view = fo_sb2[:, 0 : nt * 512].rearrange("pp (two d) -> pp two d", two=nt)
            nc.sync.dma_start(out=out_view, in_=in_view)

        # ---- pre-phase: kT j-tile 0, qT(hp0) for ic0, rest of kT(hp0) ----
        # copies split ACT/DVE so neither queue serializes ahead of the
        # first exps
        k_chunk(0, 0, nc.vector.tensor_copy)
        n_q_pre = sum(1 for c0, w in qch if c0 < IC0)
        for ci in range(n_q_pre):
            q_chunk(0, ci, nc.scalar.copy if ci % 2 == 0 else nc.vector.tensor_copy)
        for ci in range(1, len(kch)):
            k_chunk(0, ci, nc.vector.tensor_copy)

        # ---- attention ----
        sims = {}
        pts = {}

        def qk(hp, ic, j, h):
            i0 = ic * IC0
            icw = ICW[ic]
            hs = slice(h * 64, (h + 1) * 64)
            sim = psim.tile([128, icw], f32, tag="sim", name="sim")
            for c0, w in _chunks(icw):
                nc.tensor.matmul(
                    sim[:, c0 : c0 + w],
                    kT_sb[hp][hs, ts(j, 128)],
                    qT_sb[hp][hs, i0 + c0 : i0 + c0 + w],
                    start=True,
                    stop=True,
                )
            sims[(hp, ic, j, h)] = sim

        def do_exp(hp, ic, j, h):
            icw = ICW[ic]
            pt = ptp.tile([128, icw], bf16, tag="pt", name="pt")
            nc.scalar.activation(
                pt[:, :],
                sims.pop((hp, ic, j, h))[:, :],
                EXP,
                bias=bk_sb[:, j : j + 1],
                scale=1.0,
            )
            pts[(hp, ic, j, h)] = pt

        def pv(pv_ps, hp, ic, j, h):
            va = v_sb[j][:, (hp * 2 + h) * 65 : (hp * 2 + h) * 65 + 65]
            pt = pts.pop((hp, ic, j, h))
            for c0, w in _chunks(ICW[ic]):
                nc.tensor.matmul(
                    pv_ps[h][:, c0 : c0 + w],
                    va,
                    pt[:, c0 : c0 + w],
                    start=(j == 0),
                    stop=(j == njk - 1),
                )

        # per-round side work (claim-late through the "pv" psum slots)
        n_fo_side = min(4, (min(IC0, nq) // 128 + 1) // 2)
        # R2/R3 prologues read qT[1]/kT[1] chunks produced as side work, so
        # each chunk must be emitted a round before its first consumer
        n_q_pre1 = max(1, n_q_pre)
        side = {
            0: [("v", 0), ("v", 1)]
            + [("wq", 0, ci) for ci in range(n_q_pre, len(qch))]
            + [("v", j) for j in range(2, njk)],
            1: [("wq", 1, ci) for ci in range(n_q_pre1)]
            + [("wk", 1, ci) for ci in range(len(kch) - 1)],
            2: [("wk", 1, len(kch) - 1)]
            + [("wq", 1, ci) for ci in range(n_q_pre1, len(qch))],
            3: [("fo", p) for p in range(n_fo_side)],
        }
        SIDE_PER_SECTION = {0: 2, 1: 1, 2: 1, 3: 1}
        DRAIN = 4  # deferred-PV ops drained per section after the claim

        def run_side(item):
            kind = item[0]
            if kind == "v":
                v_proj(item[1])
            elif kind == "wq":
                q_chunk(item[1], item[2], nc.vector.tensor_copy)
            elif kind == "wk":
                k_chunk(item[1], item[2], nc.vector.tensor_copy)
            elif kind == "fo":
                fo_group([2 * item[1], 2 * item[1] + 1], [nc.vector.tensor_copy] * 2)

        ROUNDS = [(0, ic) for ic in range(len(ICW))] + [(1, ic) for ic in range(len(ICW))]
        for r, (hp, ic) in enumerate(ROUNDS):
            icw = ICW[ic]
            i0 = ic * IC0
            todo = list(side.get(r, []))
            per_sec = SIDE_PER_SECTION.get(r, 0)
            pv_ps = None
            pending = []

            if r == 0:
                qk(hp, ic, 0, 0)
                qk(hp, ic, 0, 1)

            for j in range(njk):
                do_exp(hp, ic, j, 0)
                do_exp(hp, ic, j, 1)
                if j + 1 < njk:
                    qk(hp, ic, j + 1, 0)
                    qk(hp, ic, j + 1, 1)
                elif r + 1 < len(ROUNDS):
                    nhp, nic = ROUNDS[r + 1]
                    qk(nhp, nic, 0, 0)
                    qk(nhp, nic, 0, 1)

                if todo:
                    for _ in range(per_sec):
                        if todo:
                            run_side(todo.pop(0))
                    pending.append((j, 0))
                    pending.append((j, 1))
                else:
                    if pv_ps is None:
                        pv_ps = [
                            ppv.tile([65, icw], f32, tag="pv", name=f"pv{h}")
                            for h in range(2)
                        ]
                    pending.append((j, 0))
                    pending.append((j, 1))
                    n = DRAIN if len(pending) > 2 else 2
                    for _ in range(min(n, len(pending))):
                        jj, hh = pending.pop(0)
                        pv(pv_ps, hp, ic, jj, hh)

            if pv_ps is None:
                pv_ps = [
                    ppv.tile([65, icw], f32, tag="pv", name=f"pv{h}") for h in range(2)
                ]
            while pending:
                jj, hh = pending.pop(0)
                pv(pv_ps, hp, ic, jj, hh)

            # normalize: aT[c, i] = pv[c, i] * (1/denom_h[i])
            last = r + 1 == len(ROUNDS)
            rr = []
            rb = []
            for h in range(2):
                rr.append(rrp.tile([1, icw], bf16, tag=f"rr{h}", name=f"rr{h}"))
                rb.append(rrp.tile([64, icw], bf16, tag=f"rb{h}", name=f"rb{h}"))

            def recip_chunk(h, c0, w):
                with nc.allow_low_precision(reason="bf16 1/denom, ~0.2% err"):
                    nc.vector.reciprocal(
                        rr[h][:, c0 : c0 + w], pv_ps[h][64:65, c0 : c0 + w]
                    )

            def norm_chunk(h, c0, w):
                nc.gpsimd.partition_broadcast(
                    rb[h][:, c0 : c0 + w], rr[h][0:1, c0 : c0 + w], channels=64
                )
                nc.vector.tensor_mul(
                    aT_sb[hp][h * 64 : (h + 1) * 64, i0 + c0 : i0 + c0 + w],
                    pv_ps[h][0:64, c0 : c0 + w],
                    rb[h][:, c0 : c0 + w],
                )

            if not last:
                for h in range(2):
                    recip_chunk(h, 0, icw)
                for h in range(2):
                    for c0, w in _chunks(icw):
                        norm_chunk(h, c0, w)
            else:
                # tail: fine-grained normalization interleaved with
                # single-tile fo units.  The hp0 partial of each fo (start=
                # True matmul) is issued before its columns are normalized --
                # it only reads aT[0], done rounds ago -- through the now-idle
                # psim slots, keeping the PE warm; the hp1 partial + copy +
                # DMA follow per-tile as the normalization sweeps.
                tail_tiles = list(range(2 * n_fo_side, njq))
                fo_ps_t = {}

                def fo_pre(it):
                    fo_ps = psim.tile([128, 512], f32, tag="sim", name="fo_ps_t")
                    nc.tensor.matmul(
                        fo_ps[:, :],
                        aT_sb[0][:, ts(it, 128)],
                        wo_sb[0][:, :],
                        start=True,
                        stop=False,
                    )
                    fo_ps_t[it] = fo_ps

                def fo_fin(it, gi):
                    fo_ps = fo_ps_t.pop(it)
                    nc.tensor.matmul(
                        fo_ps[:, :],
                        aT_sb[1][:, ts(it, 128)],
                        wo_sb[1][:, :],
                        start=False,
                        stop=True,
                    )
                    fo_sb2 = ptp.tile([128, 512], f32, tag="fo", bufs=4, name="fo_sbt")
                    eng = nc.scalar.copy if gi % 2 == 0 else nc.vector.tensor_copy
                    eng(fo_sb2[:, :], fo_ps[:, :])
                    nc.sync.dma_start(
                        out=out_d[ts(it, 128), :], in_=fo_sb2[:, :]
                    )

                for it in tail_tiles[:2]:
                    fo_pre(it)
                gi = 0
                for c0, w in _chunks(icw, 256):
                    recip_chunk(0, c0, w)
                    recip_chunk(1, c0, w)
                    norm_chunk(0, c0, w)
                    norm_chunk(1, c0, w)
                    done_cols = i0 + c0 + w
                    while gi < len(tail_tiles) and (tail_tiles[gi] + 1) * 128 <= done_cols:
                        fo_fin(tail_tiles[gi], gi)
                        if gi + 2 < len(tail_tiles):
                            fo_pre(tail_tiles[gi + 2])
                        gi += 1
                while gi < len(tail_tiles):
                    fo_fin(tail_tiles[gi], gi)
                    if gi + 2 < len(tail_tiles):
                        fo_pre(tail_tiles[gi + 2])
                    gi += 1

    nc.compile()
    return nc


def _get_nc(njk=NJ, njq=NJ):
    key = (njk, njq)
    if key not in _NC_CACHE:
        _NC_CACHE[key] = _build_nc(njk, njq)
    return _NC_CACHE[key]


def kernel(x, Wq, Wk, Wv, Wo, bo, mask_k, mask_q):
    from concourse import bass_utils

    x = np.asarray(x, np.float32)
    Wq = np.asarray(Wq, np.float32)
    Wk = np.asarray(Wk, np.float32)
    Wv = np.asarray(Wv, np.float32)
    Wo = np.asarray(Wo, np.float32)
    bo = np.asarray(bo, np.float32)
    mask_k = np.asarray(mask_k)
    mask_q = np.asarray(mask_q)

    # gather valid keys/queries to the front per batch; tile counts are the
    # worst case over batches (same program on all cores)
    korders, qorders, nvk, nvq = [], [], [], []
    for b in range(B):
        korders.append(np.argsort(~mask_k[b], kind="stable"))
        qorders.append(np.argsort(~mask_q[b], kind="stable"))
        nvk.append(int(mask_k[b].sum()))
        nvq.append(int(mask_q[b].sum()))
    njk = min(NJ, (max(nvk) + 127) // 128)
    njq = min(NJ, (max(nvq) + 127) // 128)
    nk, nq = njk * 128, njq * 128

    nc = _get_nc(njk, njq)
    scale = float(D) ** -0.5
    win_cols = WIN_BK + 2 * njk

    in_maps = []
    x_cache = {}
    for core in range(8):
        b, g = core // 2, core % 2
        cs = slice(g * CG, (g + 1) * CG)
        if b not in x_cache:
            xq = np.zeros((nq, DM), np.float32)
            tq = min(nq, N)
            xq[:tq] = x[b][qorders[b][:tq]]
            xqTb = np.ascontiguousarray(xq.T).astype(BF16)
            xk = np.zeros((nk, DM), np.float32)
            tk = min(nk, N)
            xk[:tk] = x[b][korders[b][:tk]]
            xkTb = np.ascontiguousarray(xk.T).astype(BF16)
            bkg = np.full(nk, MASK_BIAS, np.float32)
            bkg[: nvk[b]] = 0.0
            bk128 = np.ascontiguousarray(bkg.reshape(njk, 128).T)
            x_cache[b] = (xqTb, xkTb, bk128)
        xqTb, xkTb, bk128 = x_cache[b]
        win = np.zeros((DM, win_cols), BF16)
        win[:, WIN_WQ : WIN_WQ + CG] = (Wq[:, cs] * scale).astype(BF16)
        win[:, WIN_WK : WIN_WK + CG] = Wk[:, cs].astype(BF16)
        for h in range(HPG):
            win[:, WIN_WVA + h * 65 : WIN_WVA + h * 65 + 64] = Wv[
                :, g * CG + h * 64 : g * CG + (h + 1) * 64
            ].astype(BF16)
        win[0:128, WIN_BK : WIN_BK + 2 * njk] = bk128.view(BF16)
        xall = np.concatenate([xqTb, xkTb, win], axis=1)
        in_maps.append(
            {
                "xall": np.ascontiguousarray(xall),
                "wo": np.ascontiguousarray(Wo[cs, :]).astype(BF16),
            }
        )

    global _LAST_IN_MAPS
    _LAST_IN_MAPS = in_maps
    res = bass_utils.run_bass_kernel_spmd(nc, in_maps, core_ids=list(range(8)))
    outs = res.results

    out = np.empty((B, N, DM), np.float32)
    for b in range(B):
        # scatter gathered-query rows back; masked-q rows get the uniform-
        # attention fallback (reference semantics for fully-masked rows)
        dev = outs[2 * b]["out"] + outs[2 * b + 1]["out"]
        uf = (x[b].mean(0) @ Wv) @ Wo
        out[b] = uf[None, :]
        nv = nvq[b]
        out[b][qorders[b][:nv]] = dev[:nv]
        out[b] += bo[None, :]
    return out
